# revision 41
# baseline (speedup 1.0000x reference)
"""Trainium2 Bass kernel for fused attention (QKV proj + RoPE + SDPA + o_proj).

Sharding: Megatron-style tensor parallel over heads (4 heads/core x 8 cores)
for QKV+SDPA, then per-batch AllToAll rounds switch to token parallelism for
o_proj, so each core emits a disjoint slice of the final output.

Per-core schedule: software-pipelined batch loop.  QKV(b+1) chains are emitted
right after SDPA(b)'s blocks so the Tile scheduler weaves them into the PE
stalls left by the exp (ACT engine) drain of the score banks; o_proj chains
for each landed AllToAll round act as additional filler.  The attn@v
accumulation trails the scores of the next block by one block so exp results
are always ready.  QKV matmuls run in fp32r; scores/attn@v/o_proj in bf16.

RoPE uses a pre-swapped sin table so the PSUM bank is read exactly twice (two
DVE multiplies); the partition swap rides on small SBUF-to-SBUF DMAs.
"""
import sys

import numpy as np

try:
    import concourse.bass as bass
except ImportError:  # fresh grading env: make the toolchain importable
    for p in (
        "/root/.axon_site",
        "/root/.axon_site/_ro/trn_rl_repo",
        "/root/.axon_site/_ro/pypackages",
        "/opt/trn_rl_repo",
        "/opt/pypackages",
    ):
        if p not in sys.path:
            sys.path.append(p)
    import concourse.bass as bass

import ml_dtypes

import concourse.bacc as bacc
import concourse.mybir as mybir
import concourse.tile as tile
from concourse.bass_utils import run_bass_kernel_spmd

F32 = mybir.dt.float32
F32R = mybir.dt.float32r
BF16 = mybir.dt.bfloat16
MULT = mybir.AluOpType.mult
ADD = mybir.AluOpType.add
EXP = mybir.ActivationFunctionType.Exp

# problem dims (hardcoded for nn_Attention_42846593744909)
B, S, D = 4, 1024, 2048
H, HD = 32, 64
N_CORES = 8
H_LOC = H // N_CORES  # heads per core


def build_attention(b=B, s=S, d=D, h_loc=H_LOC, hd=HD, n_cores=N_CORES):
    """Build the per-core SPMD Bass program. Returns finalized nc."""
    P = 128
    T = b * s                 # total tokens (4096)
    DCH = d // P              # contraction chunks for D (16)
    QBLK = h_loc * hd         # 256: q (or k, or v) width per core
    EVA = h_loc * (hd + 1)    # v + ones columns (260)
    TCH = 512                 # qkv token chunk
    NTCB = s // TCH           # token chunks per batch (2)
    QT = 512                  # query-tile width in SDPA
    NQT = s // QT             # 2
    KTC = s // P              # key chunks of 128 per batch (8)
    ECH = n_cores * QBLK // P  # o_proj contraction chunks (16)
    SH = P                    # tokens per shard per A2A round (128)
    ODC = 256                 # o_proj dout chunk
    NDC = d // ODC            # 8

    nc = bacc.Bacc()
    hidden_t = nc.dram_tensor("hidden_t", [d, T], F32R, kind="ExternalInput")
    w_qk_t = nc.dram_tensor("w_qk_t", [d, 2 * QBLK], F32R, kind="ExternalInput")
    w_v_t = nc.dram_tensor("w_v_t", [d, QBLK], F32R, kind="ExternalInput")
    w_o_t = nc.dram_tensor("w_o_t", [n_cores * QBLK, d], BF16, kind="ExternalInput")
    cos2 = nc.dram_tensor("cos2", [P, s], BF16, kind="ExternalInput")
    sinsw2 = nc.dram_tensor("sinsw2", [P, s], BF16, kind="ExternalInput")
    out_sl = nc.dram_tensor("out_sl", [b * SH, d], F32, kind="ExternalOutput")

    hid_v = hidden_t[:].rearrange("(c p) t -> p c t", p=P)
    wqk_v = w_qk_t[:].rearrange("(c p) e -> p c e", p=P)
    wv_v = w_v_t[:].rearrange("(c p) e -> p c e", p=P)
    wo_v = w_o_t[:].rearrange("(c p) e -> p c e", p=P)

    with tile.TileContext(nc) as tc:
        with (
            tc.tile_pool(name="dramp", bufs=1, space="DRAM") as dramp,
            tc.tile_pool(name="drowp", bufs=4, space="DRAM") as drowp,
            tc.tile_pool(name="qkp", bufs=2) as qkp,
            tc.tile_pool(name="vp", bufs=2) as vp,
            tc.tile_pool(name="tabs", bufs=1) as tabs,
            tc.tile_pool(name="wqkp", bufs=1) as wqkp,
            tc.tile_pool(name="hidp", bufs=6) as hidp,
            tc.tile_pool(name="ropep", bufs=2) as ropep,
            tc.tile_pool(name="swp", bufs=2) as swp,
            tc.tile_pool(name="ep", bufs=13) as ep,
            tc.tile_pool(name="stgp", bufs=3) as stgp,
            tc.tile_pool(name="repp", bufs=3) as repp,
            tc.tile_pool(name="aop", bufs=6) as aop,
            tc.tile_pool(name="aslp", bufs=2) as aslp,
            tc.tile_pool(name="wop", bufs=2) as wop,
            tc.tile_pool(name="obp", bufs=3) as obp,
            tc.tile_pool(name="psA", bufs=2, space="PSUM") as psA,
            tc.tile_pool(name="psS", bufs=2, space="PSUM") as psS,
            tc.tile_pool(name="psO", bufs=2, space="PSUM") as psO,
        ):
            cc_in = [dramp.tile([n_cores, QBLK, SH], BF16, name=f"cc_in_{r}")
                     for r in range(b)]
            cc_out = [dramp.tile([n_cores, QBLK, SH], BF16, name=f"cc_out_{r}")
                      for r in range(b)]

            # per-dd weight tiles, one DMA each on its own queue, so the first
            # matmuls only wait ~10us for the first 256KB chunk instead of the
            # whole weight load
            wqk_c = []
            for j in range(DCH // 2):
                w = wqkp.tile([P, 2, 2 * QBLK], F32R, tag=f"wqkc{j}", name=f"wqk{j}")
                if j == 0:  # kernel-entry chunk: per-dd DMAs for latency
                    nc.sync.dma_start(w[:, 0:1], wqk_v[:, 0:1])
                    nc.sync.dma_start(w[:, 1:2], wqk_v[:, 1:2])
                else:
                    nc.sync.dma_start(w[:], wqk_v[:, 2 * j:2 * j + 2])
                wqk_c.append(w)
            wv_c = []
            for j in range(DCH // 4):
                w = wqkp.tile([P, 4, QBLK], F32R, tag=f"wvc{j}", name=f"wv{j}")
                nc.sync.dma_start(w[:], wv_v[:, 4 * j:4 * j + 4])
                wv_c.append(w)
            cos_sb = tabs.tile([P, s], BF16)
            sin_sb = tabs.tile([P, s], BF16)
            nc.sync.dma_start(cos_sb[:], cos2[:])
            nc.sync.dma_start(sin_sb[:], sinsw2[:])

            qk_t = {}   # per-batch rope'd q|k  [P, 4, s] bf16
            v_t = {}    # per-batch v (+ones)   [P, KTC, EVA] bf16
            asl = {}    # per-round o_proj activations [P, ECH, SH] bf16

            def emit_qkv(bi):
                qk_t[bi] = qkp.tile([P, 4, s], BF16, tag="qk", name=f"qk{bi}")
                v_t[bi] = vp.tile([P, KTC, EVA], BF16, tag="v", name=f"v{bi}")
                for h in range(h_loc):
                    nc.vector.memset(
                        v_t[bi][:, :, h * (hd + 1) + hd:h * (hd + 1) + hd + 1], 1.0)
                for tci in range(NTCB):
                    t0 = bi * s + tci * TCH   # global token offset
                    s0 = tci * TCH            # position within sequence
                    hidq = []
                    first = (bi == 0 and tci == 0)
                    for q in range(4):
                        hq = hidp.tile([P, 4, TCH], F32R, tag="hid",
                                       name=f"hid{bi}_{tci}_{q}")
                        if first:  # kernel-entry tiles: per-dd DMAs for latency
                            for c in range(4):
                                nc.sync.dma_start(
                                    hq[:, c:c + 1],
                                    hid_v[:, 4 * q + c:4 * q + c + 1, t0:t0 + TCH])
                        else:
                            nc.sync.dma_start(hq[:, 0:2], hid_v[:, 4 * q:4 * q + 2, t0:t0 + TCH])
                            nc.sync.dma_start(hq[:, 2:4], hid_v[:, 4 * q + 2:4 * q + 4, t0:t0 + TCH])
                        hidq.append(hq)

                    for ec in range(4):  # q0,q1,k0,k1 e-chunks
                        ps = psS.tile([P, 2 * QT], F32, tag="pss",
                                      name="psqk")[:, 0:TCH]
                        for dd in range(DCH):
                            nc.tensor.matmul(
                                ps[:], lhsT=wqk_c[dd // 2][:, dd % 2, ec * P:(ec + 1) * P],
                                rhs=hidq[dd // 4][:, dd % 4, :],
                                start=(dd == 0), stop=(dd == DCH - 1),
                            )
                        # RoPE: qk = ps*cos + swap32(ps*sin_preswapped)
                        cp = ropep.tile([P, TCH], F32, tag="cp")
                        nc.vector.tensor_tensor(cp[:], ps[:], cos_sb[:, s0:s0 + TCH], MULT)
                        tm = ropep.tile([P, TCH], F32, tag="tm")
                        nc.vector.tensor_tensor(tm[:], ps[:], sin_sb[:, s0:s0 + TCH], MULT)
                        sw = swp.tile([P, TCH], F32, tag="sw")
                        nc.sync.dma_start(sw[0:32, :], tm[32:64, :])
                        nc.sync.dma_start(sw[32:64, :], tm[0:32, :])
                        nc.sync.dma_start(sw[64:96, :], tm[96:128, :])
                        nc.sync.dma_start(sw[96:128, :], tm[64:96, :])
                        nc.vector.tensor_tensor(qk_t[bi][:, ec, s0:s0 + TCH], cp[:], sw[:], ADD)

                    for tsub in range(TCH // P):
                        kc = tci * (TCH // P) + tsub   # key chunk within batch
                        psv = psA.tile([P, QT], F32, tag="psA",
                                       name="psv")[:, 0:QBLK]
                        for dd in range(DCH):
                            nc.tensor.matmul(
                                psv[:], lhsT=hidq[dd // 4][:, dd % 4, tsub * P:(tsub + 1) * P],
                                rhs=wv_c[dd // 4][:, dd % 4, :],
                                start=(dd == 0), stop=(dd == DCH - 1),
                            )
                        nc.vector.tensor_copy(
                            v_t[bi][:, kc].rearrange("p (h e) -> p h e", h=h_loc)[:, :, 0:hd],
                            psv[:].rearrange("p (h e) -> p h e", h=h_loc),
                        )

            def emit_scores(bi, pp, qt):
                """Scores matmuls + exp for one block; returns e tiles."""
                q0 = qt * QT
                etiles = []
                for kt in range(KTC):
                    pss = psS.tile([P, 2 * QT], F32, tag="pss")
                    nc.tensor.matmul(
                        pss[:, 0:QT], lhsT=qk_t[bi][0:64, 2 + pp, kt * P:(kt + 1) * P],
                        rhs=qk_t[bi][0:64, pp, q0:q0 + QT],
                        start=True, stop=True,
                    )
                    nc.tensor.matmul(
                        pss[:, QT:2 * QT],
                        lhsT=qk_t[bi][64:128, 2 + pp, kt * P:(kt + 1) * P],
                        rhs=qk_t[bi][64:128, pp, q0:q0 + QT],
                        start=True, stop=True, tile_position=(64, 0),
                    )
                    e2 = ep.tile([P, 2 * QT], BF16, tag="e")
                    nc.scalar.activation(e2[:], pss[:], EXP)
                    etiles.append(e2)
                return etiles

            def emit_av(bi, pp, qt, etiles):
                """attn@v accumulation + normalize + cc_in writes."""
                h0, h1 = 2 * pp, 2 * pp + 1
                ps_o0 = psO.tile([P, QT], F32, tag="pso")
                ps_o1 = psO.tile([P, QT], F32, tag="pso")
                for kt in range(KTC):
                    e2 = etiles[kt]
                    nc.tensor.matmul(
                        ps_o0[0:hd + 1, :],
                        lhsT=v_t[bi][:, kt, h0 * (hd + 1):(h0 + 1) * (hd + 1)],
                        rhs=e2[:, 0:QT],
                        start=(kt == 0), stop=(kt == KTC - 1),
                    )
                    nc.tensor.matmul(
                        ps_o1[0:hd + 1, :],
                        lhsT=v_t[bi][:, kt, h1 * (hd + 1):(h1 + 1) * (hd + 1)],
                        rhs=e2[:, QT:2 * QT],
                        start=(kt == 0), stop=(kt == KTC - 1),
                    )
                # per-head softmax denominators: stage to SBUF, reciprocal,
                # bounce through DRAM to broadcast across partitions (DMA
                # partition-stride-0 reads only work from DRAM)
                # early drain: one copy frees the PSUM bank immediately;
                # reciprocal/broadcast/normalize then run from SBUF off the
                # critical path (and the ao multiply gets the 2x SBUF mode)
                for hh, ps_o in ((h0, ps_o0), (h1, ps_o1)):
                    oc = stgp.tile([hd + 1, QT], F32, tag="stg")
                    nc.vector.tensor_copy(oc[:], ps_o[0:hd + 1, :])
                    nc.vector.reciprocal(oc[hd:hd + 1, :], oc[hd:hd + 1, :])
                    rd = drowp.tile([1, QT], F32, tag="drow")
                    nc.sync.dma_start(rd[:], oc[hd:hd + 1, :])
                    rep = repp.tile([hd, QT], F32, tag="rep")
                    nc.sync.dma_start(rep[:], rd[0:1, :].to_broadcast((hd, QT)))
                    ao = aop.tile([hd, QT], BF16, tag="ao")
                    nc.vector.tensor_tensor(ao[:], oc[0:hd, :], rep[:], MULT)
                    for w in range(QT // SH):
                        j = qt * (QT // SH) + w   # shard = token_off/128
                        nc.sync.dma_start(
                            cc_in[bi][j, hh * hd:(hh + 1) * hd, :],
                            ao[:, w * SH:(w + 1) * SH],
                        )

            def emit_a2a(bi):
                nc.gpsimd.collective_compute(
                    "AllToAll",
                    mybir.AluOpType.bypass,
                    replica_groups=[list(range(n_cores))],
                    ins=[cc_in[bi].opt()],
                    outs=[cc_out[bi].opt()],
                )
                cc_v = cc_out[bi][:].rearrange("j (ci p) t -> p (j ci) t", p=P)
                asl[bi] = aslp.tile([P, ECH, SH], BF16, tag="asl", name=f"asl{bi}")
                nc.sync.dma_start(asl[bi][:], cc_v)

            def emit_oproj(bi):
                """o_proj chains for round bi's tokens (PE filler work).

                Odd rounds walk the dout chunks in reverse so the two wo
                buffers left from the previous round are reused; the final
                round draws its PSUM banks from the (now idle) psO pool.
                """
                dcs = range(NDC - 1, -1, -1) if bi % 2 else range(NDC)
                for dc in dcs:
                    wo_sb = wop.tile([P, ECH, ODC], BF16, tag="wo")
                    nc.sync.dma_start(wo_sb[:, 0:ECH // 2],
                                      wo_v[:, 0:ECH // 2, dc * ODC:(dc + 1) * ODC])
                    nc.sync.dma_start(wo_sb[:, ECH // 2:],
                                      wo_v[:, ECH // 2:, dc * ODC:(dc + 1) * ODC])
                    if bi == b - 1:
                        pso = psO.tile([P, QT], F32, tag="pso",
                                       name="pso3")[:, 0:ODC]
                    else:
                        pso = psA.tile([P, QT], F32, tag="psA",
                                       name="pso")[:, 0:ODC]
                    for e in range(ECH):
                        nc.tensor.matmul(
                            pso[:], lhsT=asl[bi][:, e, :],
                            rhs=wo_sb[:, e, :],
                            start=(e == 0), stop=(e == ECH - 1),
                        )
                    ob = obp.tile([P, ODC], F32, tag="ob")
                    nc.vector.tensor_copy(ob[:], pso[:])
                    nc.sync.dma_start(
                        out_sl[bi * SH:(bi + 1) * SH, dc * ODC:(dc + 1) * ODC],
                        ob[:],
                    )

            # batch-pipelined emission: QKV(b) before SDPA(b-1)'s trailing
            # blocks so its chains fill the exp-stall gaps of SDPA(b-1)
            blocks = [(bi, pp, qt)
                      for bi in range(b)
                      for pp in range(h_loc // 2)
                      for qt in range(NQT)]
            n_blk = (h_loc // 2) * NQT   # blocks per batch
            prev = None
            prev_e = None
            for i, blk in enumerate(blocks):
                if i % n_blk == 0:
                    emit_qkv(blk[0])
                etiles = emit_scores(*blk)
                if prev is not None:
                    emit_av(*prev, prev_e)
                    if prev[0] != blk[0]:
                        emit_a2a(prev[0])
                        emit_oproj(prev[0])
                prev, prev_e = blk, etiles
            emit_av(*prev, prev_e)
            emit_a2a(b - 1)
            emit_oproj(b - 1)
    nc.finalize()
    return nc


def prep_inputs(cos, sin, hidden_states, w_qkv, w_o,
                b=B, s=S, d=D, h_loc=H_LOC, hd=HD, n_cores=N_CORES):
    """Host-side sharding/layout: returns per-core input maps."""
    cos = np.asarray(cos, dtype=np.float32)
    sin = np.asarray(sin, dtype=np.float32)
    hidden_states = np.asarray(hidden_states, dtype=np.float32)
    w_qkv = np.asarray(w_qkv, dtype=np.float32)
    w_o = np.asarray(w_o, dtype=np.float32)

    T = b * s
    QBLK = h_loc * hd
    HHD = n_cores * QBLK  # total H*HD

    hidden_t = np.ascontiguousarray(hidden_states.reshape(T, d).T)
    w_o_t = np.ascontiguousarray(w_o.T).astype(ml_dtypes.bfloat16)

    cos_t = cos.T  # [hd, s]
    sin_t = sin.T
    cos2 = np.ascontiguousarray(np.tile(cos_t, (128 // hd, 1)))
    srt = sin_t.copy()
    srt[0:hd // 2] = -sin_t[0:hd // 2]
    srt = np.tile(srt, (128 // hd, 1))
    # pre-swap the sin table by the same 32-row block permutation the kernel's
    # swap DMAs apply: sinsw[sigma(p)] = srt[p]
    sinsw = srt.copy()
    sinsw[0:32], sinsw[32:64] = srt[32:64], srt[0:32].copy()
    sinsw[64:96], sinsw[96:128] = srt[96:128], srt[64:96].copy()
    cos2 = cos2.astype(ml_dtypes.bfloat16)
    sinsw2 = np.ascontiguousarray(sinsw).astype(ml_dtypes.bfloat16)

    maps = []
    for c in range(n_cores):
        wq = w_qkv[c * QBLK:(c + 1) * QBLK] * 0.125
        wk = w_qkv[HHD + c * QBLK:HHD + (c + 1) * QBLK]
        wv = w_qkv[2 * HHD + c * QBLK:2 * HHD + (c + 1) * QBLK]
        w_qk_t = np.ascontiguousarray(np.concatenate([wq, wk], axis=0).T)
        w_v_t = np.ascontiguousarray(wv.T)
        maps.append({
            "hidden_t": hidden_t,
            "w_qk_t": w_qk_t,
            "w_v_t": w_v_t,
            "w_o_t": w_o_t,
            "cos2": cos2,
            "sinsw2": sinsw2,
        })
    return maps


_NC_CACHE = {}


def run(inputs, trace=False, dims=None):
    """Run the distributed kernel. Returns (full_output, BassKernelResults)."""
    dims = dims or dict(b=B, s=S, d=D, h_loc=H_LOC, hd=HD, n_cores=N_CORES)
    key = tuple(sorted(dims.items()))
    if key not in _NC_CACHE:
        _NC_CACHE[key] = build_attention(**dims)
    nc = _NC_CACHE[key]
    maps = prep_inputs(inputs["cos"], inputs["sin"], inputs["hidden_states"],
                       inputs["w_qkv"], inputs["w_o"], **dims)
    res = run_bass_kernel_spmd(nc, maps, list(range(dims["n_cores"])), trace=trace)
    n_cores = dims["n_cores"]
    bb, ss, dd = dims["b"], dims["s"], dims["d"]
    SH = 128
    out = np.empty((bb, ss, dd), dtype=np.float32)
    for c in range(n_cores):
        sl = res.results[c]["out_sl"]
        for bi in range(bb):
            out[bi, c * SH:(c + 1) * SH] = sl[bi * SH:(bi + 1) * SH]
    return out, res


def kernel(**inputs) -> np.ndarray:
    out, _ = run(inputs)
    return out


# revision 44
# speedup vs baseline: 1.0708x; 1.0708x over previous
"""Trainium2 Bass kernel for fused attention (QKV proj + RoPE + SDPA + o_proj).

Sharding: Megatron-style tensor parallel over heads (4 heads/core x 8 cores)
for QKV+SDPA, then per-batch AllToAll rounds switch to token parallelism for
o_proj, so each core emits a disjoint slice of the final output.

Per-core schedule: software-pipelined batch loop.  QKV(b+1) chains are emitted
right after SDPA(b)'s blocks so the Tile scheduler weaves them into the PE
stalls left by the exp (ACT engine) drain of the score banks; o_proj chains
for each landed AllToAll round act as additional filler.  The attn@v
accumulation trails the scores of the next block by one block so exp results
are always ready.  QKV matmuls run in fp32r; scores/attn@v/o_proj in bf16.

RoPE uses a pre-swapped sin table so the PSUM bank is read exactly twice (two
DVE multiplies); the partition swap rides on small SBUF-to-SBUF DMAs.
"""
import sys

import numpy as np

try:
    import concourse.bass as bass
except ImportError:  # fresh grading env: make the toolchain importable
    for p in (
        "/root/.axon_site",
        "/root/.axon_site/_ro/trn_rl_repo",
        "/root/.axon_site/_ro/pypackages",
        "/opt/trn_rl_repo",
        "/opt/pypackages",
    ):
        if p not in sys.path:
            sys.path.append(p)
    import concourse.bass as bass

import ml_dtypes

import concourse.bacc as bacc
import concourse.mybir as mybir
import concourse.tile as tile
from concourse.bass_utils import run_bass_kernel_spmd

F32 = mybir.dt.float32
F32R = mybir.dt.float32r
BF16 = mybir.dt.bfloat16
MULT = mybir.AluOpType.mult
ADD = mybir.AluOpType.add
EXP = mybir.ActivationFunctionType.Exp

# problem dims (hardcoded for nn_Attention_42846593744909)
B, S, D = 4, 1024, 2048
H, HD = 32, 64
N_CORES = 8
H_LOC = H // N_CORES  # heads per core


def build_attention(b=B, s=S, d=D, h_loc=H_LOC, hd=HD, n_cores=N_CORES):
    """Build the per-core SPMD Bass program. Returns finalized nc."""
    P = 128
    T = b * s                 # total tokens (4096)
    DCH = d // P              # contraction chunks for D (16)
    QBLK = h_loc * hd         # 256: q (or k, or v) width per core
    EVA = h_loc * (hd + 1)    # v + ones columns (260)
    TCH = 512                 # qkv token chunk
    NTCB = s // TCH           # token chunks per batch (2)
    QT = 512                  # query-tile width in SDPA
    NQT = s // QT             # 2
    KTC = s // P              # key chunks of 128 per batch (8)
    ECH = n_cores * QBLK // P  # o_proj contraction chunks (16)
    SH = P                    # tokens per shard per A2A round (128)
    ODC = 256                 # o_proj dout chunk
    NDC = d // ODC            # 8

    nc = bacc.Bacc()
    hidden_t = nc.dram_tensor("hidden_t", [d, T], F32R, kind="ExternalInput")
    w_qk_t = nc.dram_tensor("w_qk_t", [d, 2 * QBLK], F32R, kind="ExternalInput")
    w_v_t = nc.dram_tensor("w_v_t", [d, QBLK], F32R, kind="ExternalInput")
    w_o_t = nc.dram_tensor("w_o_t", [n_cores * QBLK, d], BF16, kind="ExternalInput")
    cos2 = nc.dram_tensor("cos2", [P, s], BF16, kind="ExternalInput")
    sinsw2 = nc.dram_tensor("sinsw2", [P, s], BF16, kind="ExternalInput")
    out_sl = nc.dram_tensor("out_sl", [b * SH, d], F32, kind="ExternalOutput")

    hid_v = hidden_t[:].rearrange("(c p) t -> p c t", p=P)
    wqk_v = w_qk_t[:].rearrange("(c p) e -> p c e", p=P)
    wv_v = w_v_t[:].rearrange("(c p) e -> p c e", p=P)
    wo_v = w_o_t[:].rearrange("(c p) e -> p c e", p=P)

    with tile.TileContext(nc) as tc:
        with (
            tc.tile_pool(name="dramp", bufs=1, space="DRAM") as dramp,
            tc.tile_pool(name="drowp", bufs=4, space="DRAM") as drowp,
            tc.tile_pool(name="qkp", bufs=2) as qkp,
            tc.tile_pool(name="vp", bufs=2) as vp,
            tc.tile_pool(name="tabs", bufs=1) as tabs,
            tc.tile_pool(name="wqkp", bufs=1) as wqkp,
            tc.tile_pool(name="hidp", bufs=6) as hidp,
            tc.tile_pool(name="ropep", bufs=2) as ropep,
            tc.tile_pool(name="swp", bufs=2) as swp,
            tc.tile_pool(name="ep", bufs=12) as ep,
            tc.tile_pool(name="stgp", bufs=2) as stgp,
            tc.tile_pool(name="repp", bufs=3) as repp,
            tc.tile_pool(name="aop", bufs=6) as aop,
            tc.tile_pool(name="aslp", bufs=2) as aslp,
            tc.tile_pool(name="wop", bufs=2) as wop,
            tc.tile_pool(name="obp", bufs=3) as obp,
            tc.tile_pool(name="psA", bufs=2, space="PSUM") as psA,
            tc.tile_pool(name="psS", bufs=2, space="PSUM") as psS,
            tc.tile_pool(name="psO", bufs=2, space="PSUM") as psO,
        ):
            cc_in = [dramp.tile([n_cores, QBLK, SH], BF16, name=f"cc_in_{r}")
                     for r in range(b)]
            cc_out = [dramp.tile([n_cores, QBLK, SH], BF16, name=f"cc_out_{r}")
                      for r in range(b)]

            # per-dd weight tiles, one DMA each on its own queue, so the first
            # matmuls only wait ~10us for the first 256KB chunk instead of the
            # whole weight load
            wqk_c = []
            for j in range(DCH // 2):
                w = wqkp.tile([P, 2, 2 * QBLK], F32R, tag=f"wqkc{j}", name=f"wqk{j}")
                if j == 0:  # kernel-entry chunk: per-dd DMAs for latency
                    nc.sync.dma_start(w[:, 0:1], wqk_v[:, 0:1])
                    nc.sync.dma_start(w[:, 1:2], wqk_v[:, 1:2])
                else:
                    nc.sync.dma_start(w[:], wqk_v[:, 2 * j:2 * j + 2])
                wqk_c.append(w)
            wv_c = []
            for j in range(DCH // 4):
                w = wqkp.tile([P, 4, QBLK], F32R, tag=f"wvc{j}", name=f"wv{j}")
                nc.sync.dma_start(w[:], wv_v[:, 4 * j:4 * j + 4])
                wv_c.append(w)
            cos_sb = tabs.tile([P, s], BF16)
            sin_sb = tabs.tile([P, s], BF16)
            nc.sync.dma_start(cos_sb[:], cos2[:])
            nc.sync.dma_start(sin_sb[:], sinsw2[:])

            qk_t = {}   # per-batch rope'd q|k  [P, 4, s] bf16
            v_t = {}    # per-batch v (+ones)   [P, KTC, EVA] bf16
            asl = {}    # per-round o_proj activations [P, ECH, SH] bf16

            def emit_qkv(bi):
                qk_t[bi] = qkp.tile([P, 4, s], BF16, tag="qk", name=f"qk{bi}")
                v_t[bi] = vp.tile([P, KTC, EVA], BF16, tag="v", name=f"v{bi}")
                for h in range(h_loc):
                    nc.vector.memset(
                        v_t[bi][:, :, h * (hd + 1) + hd:h * (hd + 1) + hd + 1], 1.0)
                for tci in range(NTCB):
                    t0 = bi * s + tci * TCH   # global token offset
                    s0 = tci * TCH            # position within sequence
                    hidq = []
                    first = (bi == 0 and tci == 0)
                    for q in range(4):
                        hq = hidp.tile([P, 4, TCH], F32R, tag="hid",
                                       name=f"hid{bi}_{tci}_{q}")
                        if first:  # kernel-entry tiles: per-dd DMAs for latency
                            for c in range(4):
                                nc.sync.dma_start(
                                    hq[:, c:c + 1],
                                    hid_v[:, 4 * q + c:4 * q + c + 1, t0:t0 + TCH])
                        else:
                            nc.sync.dma_start(hq[:, 0:2], hid_v[:, 4 * q:4 * q + 2, t0:t0 + TCH])
                            nc.sync.dma_start(hq[:, 2:4], hid_v[:, 4 * q + 2:4 * q + 4, t0:t0 + TCH])
                        hidq.append(hq)

                    for ec in range(4):  # q0,q1,k0,k1 e-chunks
                        ps = psS.tile([P, 2 * QT], F32, tag="pss",
                                      name="psqk")[:, 0:TCH]
                        for dd in range(DCH):
                            nc.tensor.matmul(
                                ps[:], lhsT=wqk_c[dd // 2][:, dd % 2, ec * P:(ec + 1) * P],
                                rhs=hidq[dd // 4][:, dd % 4, :],
                                start=(dd == 0), stop=(dd == DCH - 1),
                            )
                        # RoPE: qk = ps*cos + swap32(ps*sin_preswapped)
                        cp = ropep.tile([P, TCH], F32, tag="cp")
                        nc.vector.tensor_tensor(cp[:], ps[:], cos_sb[:, s0:s0 + TCH], MULT)
                        tm = ropep.tile([P, TCH], F32, tag="tm")
                        nc.vector.tensor_tensor(tm[:], ps[:], sin_sb[:, s0:s0 + TCH], MULT)
                        sw = swp.tile([P, TCH], F32, tag="sw")
                        nc.sync.dma_start(sw[0:32, :], tm[32:64, :])
                        nc.sync.dma_start(sw[32:64, :], tm[0:32, :])
                        nc.sync.dma_start(sw[64:96, :], tm[96:128, :])
                        nc.sync.dma_start(sw[96:128, :], tm[64:96, :])
                        nc.vector.tensor_tensor(qk_t[bi][:, ec, s0:s0 + TCH], cp[:], sw[:], ADD)

                    for tsub in range(TCH // P):
                        kc = tci * (TCH // P) + tsub   # key chunk within batch
                        psv = psA.tile([P, QT], F32, tag="psA",
                                       name="psv")[:, 0:QBLK]
                        for dd in range(DCH):
                            nc.tensor.matmul(
                                psv[:], lhsT=hidq[dd // 4][:, dd % 4, tsub * P:(tsub + 1) * P],
                                rhs=wv_c[dd // 4][:, dd % 4, :],
                                start=(dd == 0), stop=(dd == DCH - 1),
                            )
                        nc.vector.tensor_copy(
                            v_t[bi][:, kc].rearrange("p (h e) -> p h e", h=h_loc)[:, :, 0:hd],
                            psv[:].rearrange("p (h e) -> p h e", h=h_loc),
                        )

            def emit_scores(bi, pp, qt):
                """Scores matmuls + exp for one block; returns e tiles."""
                q0 = qt * QT
                etiles = []
                for kt in range(KTC):
                    pss = psS.tile([P, 2 * QT], F32, tag="pss")
                    nc.tensor.matmul(
                        pss[:, 0:QT], lhsT=qk_t[bi][0:64, 2 + pp, kt * P:(kt + 1) * P],
                        rhs=qk_t[bi][0:64, pp, q0:q0 + QT],
                        start=True, stop=True,
                    )
                    nc.tensor.matmul(
                        pss[:, QT:2 * QT],
                        lhsT=qk_t[bi][64:128, 2 + pp, kt * P:(kt + 1) * P],
                        rhs=qk_t[bi][64:128, pp, q0:q0 + QT],
                        start=True, stop=True, tile_position=(64, 0),
                    )
                    e2 = ep.tile([P, 2 * QT], BF16, tag="e")
                    nc.scalar.activation(e2[:], pss[:], EXP)
                    etiles.append(e2)
                return etiles

            def emit_av(bi, pp, qt, etiles):
                """attn@v accumulation + normalize + cc_in writes."""
                h0, h1 = 2 * pp, 2 * pp + 1
                ps_o0 = psO.tile([P, QT], F32, tag="pso")
                ps_o1 = psO.tile([P, QT], F32, tag="pso")
                for kt in range(KTC):
                    e2 = etiles[kt]
                    nc.tensor.matmul(
                        ps_o0[0:hd + 1, :],
                        lhsT=v_t[bi][:, kt, h0 * (hd + 1):(h0 + 1) * (hd + 1)],
                        rhs=e2[:, 0:QT],
                        start=(kt == 0), stop=(kt == KTC - 1),
                    )
                    nc.tensor.matmul(
                        ps_o1[0:hd + 1, :],
                        lhsT=v_t[bi][:, kt, h1 * (hd + 1):(h1 + 1) * (hd + 1)],
                        rhs=e2[:, QT:2 * QT],
                        start=(kt == 0), stop=(kt == KTC - 1),
                    )
                # per-head softmax denominators: stage to SBUF, reciprocal,
                # bounce through DRAM to broadcast across partitions (DMA
                # partition-stride-0 reads only work from DRAM)
                # early drain: one copy frees the PSUM bank immediately;
                # reciprocal/broadcast/normalize then run from SBUF off the
                # critical path (and the ao multiply gets the 2x SBUF mode)
                for hh, ps_o in ((h0, ps_o0), (h1, ps_o1)):
                    oc = stgp.tile([hd + 1, QT], F32, tag="stg")
                    nc.vector.tensor_copy(oc[:], ps_o[0:hd + 1, :])
                    nc.vector.reciprocal(oc[hd:hd + 1, :], oc[hd:hd + 1, :])
                    rd = drowp.tile([1, QT], F32, tag="drow")
                    nc.sync.dma_start(rd[:], oc[hd:hd + 1, :])
                    rep = repp.tile([hd, QT], F32, tag="rep")
                    nc.sync.dma_start(rep[:], rd[0:1, :].to_broadcast((hd, QT)))
                    ao = aop.tile([hd, QT], BF16, tag="ao")
                    nc.vector.tensor_tensor(ao[:], oc[0:hd, :], rep[:], MULT)
                    for w in range(QT // SH):
                        j = qt * (QT // SH) + w   # shard = token_off/128
                        nc.sync.dma_start(
                            cc_in[bi][j, hh * hd:(hh + 1) * hd, :],
                            ao[:, w * SH:(w + 1) * SH],
                        )

            def emit_a2a(bi):
                nc.gpsimd.collective_compute(
                    "AllToAll",
                    mybir.AluOpType.bypass,
                    replica_groups=[list(range(n_cores))],
                    ins=[cc_in[bi].opt()],
                    outs=[cc_out[bi].opt()],
                )
                cc_v = cc_out[bi][:].rearrange("j (ci p) t -> p (j ci) t", p=P)
                asl[bi] = aslp.tile([P, ECH, SH], BF16, tag="asl", name=f"asl{bi}")
                nc.sync.dma_start(asl[bi][:], cc_v)

            def emit_oproj(bi):
                """o_proj chains for round bi's tokens (PE filler work).

                Odd rounds walk the dout chunks in reverse so the two wo
                buffers left from the previous round are reused; the final
                round draws its PSUM banks from the (now idle) psO pool.
                """
                dcs = range(NDC - 1, -1, -1) if bi % 2 else range(NDC)
                for dc in dcs:
                    wo_sb = wop.tile([P, ECH, ODC], BF16, tag="wo")
                    nc.sync.dma_start(wo_sb[:, 0:ECH // 2],
                                      wo_v[:, 0:ECH // 2, dc * ODC:(dc + 1) * ODC])
                    nc.sync.dma_start(wo_sb[:, ECH // 2:],
                                      wo_v[:, ECH // 2:, dc * ODC:(dc + 1) * ODC])
                    if bi == b - 1:
                        pso = psO.tile([P, QT], F32, tag="pso",
                                       name="pso3")[:, 0:ODC]
                    else:
                        pso = psA.tile([P, QT], F32, tag="psA",
                                       name="pso")[:, 0:ODC]
                    for e in range(ECH):
                        nc.tensor.matmul(
                            pso[:], lhsT=asl[bi][:, e, :],
                            rhs=wo_sb[:, e, :],
                            start=(e == 0), stop=(e == ECH - 1),
                        )
                    ob = obp.tile([P, ODC], F32, tag="ob")
                    nc.vector.tensor_copy(ob[:], pso[:])
                    nc.sync.dma_start(
                        out_sl[bi * SH:(bi + 1) * SH, dc * ODC:(dc + 1) * ODC],
                        ob[:],
                    )

            # batch-pipelined emission: QKV(b) before SDPA(b-1)'s trailing
            # blocks so its chains fill the exp-stall gaps of SDPA(b-1)
            blocks = [(bi, pp, qt)
                      for bi in range(b)
                      for pp in range(h_loc // 2)
                      for qt in range(NQT)]
            n_blk = (h_loc // 2) * NQT   # blocks per batch
            prev = None
            prev_e = None
            for i, blk in enumerate(blocks):
                if i % n_blk == 0:
                    emit_qkv(blk[0])
                etiles = emit_scores(*blk)
                if prev is not None:
                    emit_av(*prev, prev_e)
                    if prev[0] != blk[0]:
                        emit_a2a(prev[0])
                        emit_oproj(prev[0])
                prev, prev_e = blk, etiles
            emit_av(*prev, prev_e)
            emit_a2a(b - 1)
            emit_oproj(b - 1)
    nc.finalize()
    return nc


def prep_inputs(cos, sin, hidden_states, w_qkv, w_o,
                b=B, s=S, d=D, h_loc=H_LOC, hd=HD, n_cores=N_CORES):
    """Host-side sharding/layout: returns per-core input maps."""
    cos = np.asarray(cos, dtype=np.float32)
    sin = np.asarray(sin, dtype=np.float32)
    hidden_states = np.asarray(hidden_states, dtype=np.float32)
    w_qkv = np.asarray(w_qkv, dtype=np.float32)
    w_o = np.asarray(w_o, dtype=np.float32)

    T = b * s
    QBLK = h_loc * hd
    HHD = n_cores * QBLK  # total H*HD

    hidden_t = np.ascontiguousarray(hidden_states.reshape(T, d).T)
    w_o_t = np.ascontiguousarray(w_o.T).astype(ml_dtypes.bfloat16)

    cos_t = cos.T  # [hd, s]
    sin_t = sin.T
    cos2 = np.ascontiguousarray(np.tile(cos_t, (128 // hd, 1)))
    srt = sin_t.copy()
    srt[0:hd // 2] = -sin_t[0:hd // 2]
    srt = np.tile(srt, (128 // hd, 1))
    # pre-swap the sin table by the same 32-row block permutation the kernel's
    # swap DMAs apply: sinsw[sigma(p)] = srt[p]
    sinsw = srt.copy()
    sinsw[0:32], sinsw[32:64] = srt[32:64], srt[0:32].copy()
    sinsw[64:96], sinsw[96:128] = srt[96:128], srt[64:96].copy()
    cos2 = cos2.astype(ml_dtypes.bfloat16)
    sinsw2 = np.ascontiguousarray(sinsw).astype(ml_dtypes.bfloat16)

    maps = []
    for c in range(n_cores):
        wq = w_qkv[c * QBLK:(c + 1) * QBLK] * 0.125
        wk = w_qkv[HHD + c * QBLK:HHD + (c + 1) * QBLK]
        wv = w_qkv[2 * HHD + c * QBLK:2 * HHD + (c + 1) * QBLK]
        w_qk_t = np.ascontiguousarray(np.concatenate([wq, wk], axis=0).T)
        w_v_t = np.ascontiguousarray(wv.T)
        maps.append({
            "hidden_t": hidden_t,
            "w_qk_t": w_qk_t,
            "w_v_t": w_v_t,
            "w_o_t": w_o_t,
            "cos2": cos2,
            "sinsw2": sinsw2,
        })
    return maps


_NC_CACHE = {}


def run(inputs, trace=False, dims=None):
    """Run the distributed kernel. Returns (full_output, BassKernelResults)."""
    dims = dims or dict(b=B, s=S, d=D, h_loc=H_LOC, hd=HD, n_cores=N_CORES)
    key = tuple(sorted(dims.items()))
    if key not in _NC_CACHE:
        _NC_CACHE[key] = build_attention(**dims)
    nc = _NC_CACHE[key]
    maps = prep_inputs(inputs["cos"], inputs["sin"], inputs["hidden_states"],
                       inputs["w_qkv"], inputs["w_o"], **dims)
    res = run_bass_kernel_spmd(nc, maps, list(range(dims["n_cores"])), trace=trace)
    n_cores = dims["n_cores"]
    bb, ss, dd = dims["b"], dims["s"], dims["d"]
    SH = 128
    out = np.empty((bb, ss, dd), dtype=np.float32)
    for c in range(n_cores):
        sl = res.results[c]["out_sl"]
        for bi in range(bb):
            out[bi, c * SH:(c + 1) * SH] = sl[bi * SH:(bi + 1) * SH]
    return out, res


def kernel(**inputs) -> np.ndarray:
    out, _ = run(inputs)
    return out
